# revision 8
# baseline (speedup 1.0000x reference)
"""BiLSTM + classifier + CRF (loss, logits, viterbi tags) on 8 Trainium2 cores.

Strategy:
  Launch 1 (LSTM): cores 0-3 run the forward-direction LSTM for 16 sequences
  each; cores 4-7 run the backward direction (host supplies length-reversed
  sequences). Input projection (with bias and mask folded in as augmented
  features) is precomputed to DRAM, then a 512-step recurrence streams
  W_hh^T through the PE in float32r. Each core also applies its half of the
  classifier on the fly, emitting per-direction logit partials.
  Host: un-reverse the backward partials, sum halves + bias -> logits.

  Launch 2 (CRF): 8 cores x 8 sequences. Per step: CRF forward recurrence
  via exp/matmul/log (logsumexp as a real PE matmul against exp(trans)) and
  Viterbi max-plus recurrence on the vector engine, storing the score
  history. Host: numerator, denominator, loss, and Viterbi backtrace from
  the score history.

Masked (padded) steps are handled exactly for prefix masks by forcing
f-gate -> 1, i/o-gates -> 0 via a rank-1 augmentation of the input
projection; non-prefix masks fall back to a numpy implementation.
"""

import numpy as np

import concourse.bass as bass
import concourse.mybir as mybir
from concourse.bass import ds
from concourse.bass_utils import run_bass_kernel_spmd
from concourse.tile import TileContext
from bass_rust import ScopedClock

F32 = mybir.dt.float32
F32R = mybir.dt.float32r
AF = mybir.ActivationFunctionType
ALU = mybir.AluOpType

B, S, I, H, T = 64, 512, 300, 512, 25
NCORES = 8
BC1 = 16          # sequences per core, launch 1
BC2 = 8           # sequences per core, launch 2
KAUG = 384        # padded augmented feature dim (300 features + 1 + mask + pad)
G4 = 4 * H        # 2048 gates
BIG = 1.0e4


class SplitDrainTileContext(TileContext):
    """TileContext whose tail drain splits sem waits one-per-instruction
    (walrus codegen caps sync waits per instruction)."""

    def _legalize_multi_waits(self):
        """Hoist extra sem waits onto same-engine nops ahead of any
        instruction carrying more than one wait (HW wait-slot limit)."""
        nc = self.nc
        cnt = 0
        for f in nc.m.functions:
            for b in f.blocks:
                insts = list(b.instructions)
                if not any(
                    i.sync_info is not None and i.sync_info.on_wait
                    and len(i.sync_info.on_wait) > 1 for i in insts
                ):
                    continue
                out = []
                for i in insts:
                    si = i.sync_info
                    if si is not None and si.on_wait and len(si.on_wait) > 1:
                        waits = list(si.on_wait)
                        for w in waits[:-1]:
                            nop = mybir.InstNoOp(name=f"legalwait-{cnt}")
                            cnt += 1
                            nop.engine = i.engine
                            nop.sync_info = mybir.SyncInfo(
                                on_wait=[w], on_update=[])
                            nc.register_instruction(nop)
                            out.append(nop)
                        si.on_wait = waits[-1:]
                    out.append(i)
                b.instructions = out

    def _drain_and_barrier(self, tick_clock, wait_clock):
        nc = self.nc
        drain_inst = nc.sync.drain()
        wait_clock.add_sem_waits(
            drain_inst.ins, ScopedClock({None: tick_clock.global_clock})
        )
        si = drain_inst.ins.sync_info
        if si is not None and si.on_wait and len(si.on_wait) > 1:
            waits = list(si.on_wait)
            si.on_wait = waits[:1]
            for w in waits[1:]:
                nop = nc.sync.nop()
                nop.ins.sync_info = type(si)(on_wait=[w], on_update=[])
        self._legalize_multi_waits()
        nc.all_engine_barrier()
        assert self.sems is not None
        popped = nc._tile_sem_poison_stack.pop()
        assert popped is self._sem_poison
        nc.clear_and_free_semaphores(list(self.sems.allocated().values()))
        nc.all_engine_barrier()


# ---------------------------------------------------------------- launch 1

def build_lstm(s):
    """Per-core program: input-projection prologue + LSTM recurrence +
    classifier half. Identical on all cores; direction differences live in
    the data each core receives."""
    nt = (BC1 * s) // 128          # 128-token prologue tiles
    ng = s // 32                   # 32-step classifier groups
    nc = bass.Bass("TRN2", target_bir_lowering=False, debug=False,
                   num_devices=NCORES)
    xt = nc.dram_tensor("xt", [nt, KAUG, 128], F32, kind="ExternalInput").ap()
    waT = nc.dram_tensor("waT", [KAUG, G4], F32, kind="ExternalInput").ap()
    whT = nc.dram_tensor("whT", [H, G4], F32, kind="ExternalInput").ap()
    wcT = nc.dram_tensor("wcT", [H, T], F32, kind="ExternalInput").ap()
    idf = nc.dram_tensor("idf", [128, 128], F32, kind="ExternalInput").ap()
    pre = nc.dram_tensor("pre", [s * BC1, G4], F32).ap()
    lfT = nc.dram_tensor("lfT", [ng, T, 512], F32, kind="ExternalOutput").ap()

    with SplitDrainTileContext(nc) as tc:
        with tc.tile_pool(name="const", bufs=1) as cpool:
            wh_t = []
            for k in range(4):
                t_ = cpool.tile([128, G4], F32R, tag=f"wh{k}")
                nc.gpsimd.dma_start(out=t_[:], in_=whT[128 * k:128 * (k + 1), :])
                wh_t.append(t_)
            wc_t = []
            for k in range(4):
                t_ = cpool.tile([128, T], F32R, tag=f"wc{k}")
                nc.gpsimd.dma_start(out=t_[:], in_=wcT[128 * k:128 * (k + 1), :])
                wc_t.append(t_)
            id16r = cpool.tile([16, 16], F32R, tag="id16r")
            nc.gpsimd.dma_start(out=id16r[:], in_=idf[0:16, 0:16])
            idf_t = cpool.tile([128, 128], F32, tag="idf")
            nc.sync.dma_start(out=idf_t[:], in_=idf[:])

            # -------- prologue: pre = x_aug @ W_aug^T  (token-tiled)
            with (
                tc.tile_pool(name="wa", bufs=1) as wapool,
                tc.tile_pool(name="prolog", bufs=3) as ppool,
                tc.tile_pool(name="pps", bufs=2, space="PSUM") as pps,
            ):
                wa_t = []
                for k in range(3):
                    t_ = wapool.tile([128, G4], F32, tag=f"wa{k}")
                    nc.sync.dma_start(out=t_[:], in_=waT[128 * k:128 * (k + 1), :])
                    wa_t.append(t_)
                for g in range(nt):
                    xts = []
                    for k in range(3):
                        t_ = ppool.tile([128, 128], F32, tag=f"xt{k}")
                        nc.sync.dma_start(
                            out=t_[:], in_=xt[g, 128 * k:128 * (k + 1), :])
                        xts.append(t_)
                    ps = pps.tile([128, G4], F32, tag="ps")
                    for c in range(4):
                        for k in range(3):
                            nc.tensor.matmul(
                                ps[:, ds(512 * c, 512)], xts[k][:],
                                wa_t[k][:, ds(512 * c, 512)],
                                start=(k == 0), stop=(k == 2))
                    pss = ppool.tile([128, G4], F32, tag="pss")
                    nc.vector.tensor_copy(pss[:], ps[:])
                    nc.sync.dma_start(
                        out=pre[128 * g:128 * (g + 1), :], in_=pss[:])

            # -------- recurrence
            with (
                tc.tile_pool(name="rec", bufs=2) as rpool,
                tc.tile_pool(name="prer", bufs=4) as prepool,
                tc.tile_pool(name="hT", bufs=2) as hpool,
                tc.tile_pool(name="gps", bufs=1, space="PSUM") as gpsum,
                tc.tile_pool(name="tps", bufs=2, space="PSUM") as tpsum,
                tc.tile_pool(name="lfp", bufs=2, space="PSUM") as lfpsum,
            ):
                czero = rpool.tile([16, 512], F32, tag="c0")
                nc.gpsimd.memset(czero[:], 0.0)
                c_prev = czero
                h_prev_buf = None
                h_prev_off = 0
                hTbuf = None
                for t in range(s):
                    t32 = t % 32
                    if t32 == 0:
                        hTbuf = hpool.tile([128, G4], F32R, tag="hTb")
                    pre_t = prepool.tile([16, G4], F32, tag="pre")
                    nc.sync.dma_start(
                        out=pre_t[:], in_=pre[16 * t:16 * (t + 1), :])
                    if t > 0:
                        gates = gpsum.tile([16, G4], F32, tag="g")
                        for c in range(4):
                            cs = ds(512 * c, 512)
                            for k in range(4):
                                nc.tensor.matmul(
                                    gates[:, cs],
                                    h_prev_buf[:, ds(512 * k + 16 * h_prev_off, 16)],
                                    wh_t[k][:, cs],
                                    start=(k == 0), stop=(k == 3))
                        gsum = rpool.tile([16, G4], F32, tag="gsum")
                        nc.vector.scalar_tensor_tensor(
                            gsum[:], gates[:], 1.0, pre_t[:], ALU.mult, ALU.add)
                    else:
                        gsum = pre_t
                    sg = rpool.tile([16, 1536], F32, tag="sg")
                    nc.scalar.activation(sg[:], gsum[:, 0:1536], AF.Sigmoid)
                    tg = rpool.tile([16, 512], F32, tag="tg")
                    nc.scalar.activation(tg[:], gsum[:, 1536:2048], AF.Tanh)
                    t1 = rpool.tile([16, 512], F32, tag="t1")
                    nc.vector.tensor_tensor(t1[:], sg[:, 0:512], tg[:], ALU.mult)
                    t2 = rpool.tile([16, 512], F32, tag="t2")
                    nc.vector.tensor_tensor(t2[:], sg[:, 512:1024], c_prev[:],
                                            ALU.mult)
                    c_new = rpool.tile([16, 512], F32, tag="c")
                    nc.vector.tensor_tensor(c_new[:], t1[:], t2[:], ALU.add)
                    tcn = rpool.tile([16, 512], F32, tag="tc")
                    nc.scalar.activation(tcn[:], c_new[:], AF.Tanh)
                    h_new = rpool.tile([16, 512], F32, tag="h")
                    nc.vector.tensor_tensor(h_new[:], sg[:, 1024:1536], tcn[:],
                                            ALU.mult)
                    tp = tpsum.tile([128, 64], F32, tag="tp")
                    for k in range(4):
                        nc.tensor.transpose(
                            tp[:, ds(16 * k, 16)],
                            h_new[:, ds(128 * k, 128)], idf_t[0:16, 0:16])
                    hT_view = hTbuf[:].rearrange(
                        "p (k m) -> p k m", k=4)[:, :, ds(16 * t32, 16)]
                    tp_view = tp[:].rearrange("p (k m) -> p k m", k=4)
                    nc.vector.tensor_copy(hT_view, tp_view)
                    c_prev = c_new
                    h_prev_buf = hTbuf
                    h_prev_off = t32
                    if t32 == 31:
                        g32 = t // 32
                        lfp = lfpsum.tile([T, 512], F32, tag="lfp")
                        for k in range(4):
                            nc.tensor.matmul(
                                lfp[:], wc_t[k][:],
                                hTbuf[:, ds(512 * k, 512)],
                                start=(k == 0), stop=(k == 3))
                        lfs = rpool.tile([T, 512], F32, tag="lfs")
                        nc.vector.tensor_copy(lfs[:], lfp[:])
                        nc.sync.dma_start(out=lfT[g32], in_=lfs[:])
    return nc


# ---------------------------------------------------------------- launch 2

def build_crf(s):
    """Per-core program: CRF forward recurrence (logsumexp via PE matmul
    against exp(trans)) + Viterbi max-plus forward storing score history."""
    nch = s // 64                  # 64-step history chunks
    nc = bass.Bass("TRN2", target_bir_lowering=False, debug=False,
                   num_devices=NCORES)
    e = nc.dram_tensor("e", [BC2, s, T], F32, kind="ExternalInput").ap()
    ttr = nc.dram_tensor("ttr", [BC2, T * T], F32, kind="ExternalInput").ap()
    etr = nc.dram_tensor("etr", [T, T], F32, kind="ExternalInput").ap()
    mask = nc.dram_tensor("mask", [BC2, s], F32, kind="ExternalInput").ap()
    idf = nc.dram_tensor("idf", [128, 128], F32, kind="ExternalInput").ap()
    vhist = nc.dram_tensor("vhist", [BC2, s, T], F32, kind="ExternalOutput").ap()
    crf = nc.dram_tensor("crf", [BC2, T], F32, kind="ExternalOutput").ap()

    with SplitDrainTileContext(nc) as tc:
        with (
            tc.tile_pool(name="const", bufs=1) as cpool,
            tc.tile_pool(name="er", bufs=3) as epool,
            tc.tile_pool(name="vr", bufs=3) as vpool,
            tc.tile_pool(name="sc", bufs=2) as spool,
            tc.tile_pool(name="ps2", bufs=2, space="PSUM") as ps2,
        ):
            ttr_t = cpool.tile([BC2, T * T], F32, tag="ttr")
            nc.sync.dma_start(out=ttr_t[:], in_=ttr[:])
            etr_t = cpool.tile([T, T], F32R, tag="etr")
            nc.gpsimd.dma_start(out=etr_t[:], in_=etr[:])
            mask_t = cpool.tile([BC2, s], F32, tag="mask")
            nc.sync.dma_start(out=mask_t[:], in_=mask[:])
            idf_t = cpool.tile([128, 128], F32, tag="idf")
            nc.sync.dma_start(out=idf_t[:], in_=idf[:])

            e_buf = None
            v_buf = None
            v_prev = None
            crf_prev = None
            for t in range(s):
                tc64 = t % 64
                ch = t // 64
                if tc64 == 0:
                    e_buf = epool.tile([BC2, 64 * T], F32, tag="eb")
                    nc.sync.dma_start(out=e_buf[:], in_=e[:, ds(64 * ch, 64), :])
                    v_buf = vpool.tile([BC2, 64 * T], F32, tag="vb")
                e_sl = e_buf[:, ds(T * tc64, T)]
                v_sl = v_buf[:, ds(T * tc64, T)]
                if t == 0:
                    nc.vector.tensor_copy(v_sl, e_sl)
                    c0 = spool.tile([BC2, T], F32, tag="crf")
                    nc.vector.tensor_copy(c0[:], e_sl)
                    v_prev = v_sl
                    crf_prev = c0
                else:
                    m_sl = mask_t[:, ds(t, 1)]
                    # ---- Viterbi (max-plus) step
                    cand = spool.tile([BC2, T * T], F32, tag="cand")
                    nc.vector.tensor_tensor(
                        cand[:],
                        v_prev.unsqueeze(1).broadcast_to([BC2, T, T]),
                        ttr_t[:].rearrange("p (j i) -> p j i", j=T),
                        ALU.add)
                    best = spool.tile([BC2, T], F32, tag="best")
                    nc.vector.tensor_reduce(
                        best[:],
                        cand[:].rearrange("p (j i) -> p j i", j=T),
                        mybir.AxisListType.X, ALU.max)
                    vt = spool.tile([BC2, T], F32, tag="vt")
                    nc.vector.tensor_tensor(vt[:], best[:], e_sl, ALU.add)
                    vd = spool.tile([BC2, T], F32, tag="vd")
                    nc.vector.tensor_tensor(vd[:], vt[:], v_prev, ALU.subtract)
                    nc.vector.scalar_tensor_tensor(
                        v_sl, vd[:], m_sl, v_prev, ALU.mult, ALU.add)
                    v_prev = v_sl
                    # ---- CRF forward (logsumexp) step
                    mneg = spool.tile([BC2, 1], F32, tag="mneg")
                    nc.vector.tensor_reduce(
                        mneg[:], crf_prev[:], mybir.AxisListType.X, ALU.max,
                        negate=True)
                    exps = spool.tile([BC2, T], F32, tag="exps")
                    nc.scalar.activation(exps[:], crf_prev[:], AF.Exp,
                                         bias=mneg[:])
                    expsT_p = ps2.tile([T, BC2], F32, tag="expsTp")
                    nc.tensor.transpose(expsT_p[:], exps[:],
                                        idf_t[0:BC2, 0:BC2])
                    expsT = spool.tile([T, BC2], F32R, tag="expsT")
                    nc.vector.tensor_copy(expsT[:], expsT_p[:])
                    acc = ps2.tile([T, BC2], F32, tag="acc")
                    nc.tensor.matmul(acc[:], etr_t[:], expsT[:],
                                     start=True, stop=True)
                    lnacc = spool.tile([T, BC2], F32, tag="lnacc")
                    nc.scalar.activation(lnacc[:], acc[:], AF.Ln)
                    lnT = ps2.tile([BC2, T], F32, tag="lnT")
                    nc.tensor.transpose(lnT[:], lnacc[:], idf_t[0:T, 0:T])
                    ctmp = spool.tile([BC2, T], F32, tag="ctmp")
                    nc.vector.scalar_tensor_tensor(
                        ctmp[:], lnT[:], mneg[:], e_sl, ALU.subtract, ALU.add)
                    cd = spool.tile([BC2, T], F32, tag="cd")
                    nc.vector.tensor_tensor(cd[:], ctmp[:], crf_prev[:],
                                            ALU.subtract)
                    crf_new = spool.tile([BC2, T], F32, tag="crf")
                    nc.vector.scalar_tensor_tensor(
                        crf_new[:], cd[:], m_sl, crf_prev[:], ALU.mult, ALU.add)
                    crf_prev = crf_new
                if tc64 == 63 or t == s - 1:
                    nc.sync.dma_start(out=vhist[:, ds(64 * ch, 64), :],
                                      in_=v_buf[:])
            nc.sync.dma_start(out=crf[:], in_=crf_prev[:])
    return nc


# ---------------------------------------------------------------- host side

_GATE_PERM = None


def _gate_perm():
    global _GATE_PERM
    if _GATE_PERM is None:
        # pytorch gate order i,f,g,o -> our layout [i, f, o, g]
        _GATE_PERM = np.concatenate([
            np.arange(0, H), np.arange(H, 2 * H),
            np.arange(3 * H, 4 * H), np.arange(2 * H, 3 * H)])
    return _GATE_PERM


def _build_waT(W_ih, b):
    p = _gate_perm()
    wa = np.zeros((KAUG, G4), np.float32)
    wa[0:I, :] = W_ih[p].T.astype(np.float32)
    wa[I, :] = b[p].astype(np.float32)
    mv = np.zeros(G4, np.float32)
    mv[0:H] = -BIG           # i
    mv[H:2 * H] = BIG        # f
    mv[2 * H:3 * H] = -BIG   # o
    mv[3 * H:] = 0.0         # g
    wa[I + 1, :] = mv
    return wa


def _build_xt(x_core, mask_core, s):
    """x_core [16, s, I] f32, mask_core [16, s] -> [nt, KAUG, 128]."""
    xa = np.zeros((BC1, s, KAUG), np.float32)
    xa[:, :, 0:I] = x_core
    xa[:, :, I] = 1.0
    xa[:, :, I + 1] = 1.0 - mask_core
    # tile g covers steps 8g..8g+8; col = b + 16*t8
    v = xa.reshape(BC1, s // 8, 8, KAUG).transpose(1, 3, 2, 0)
    return np.ascontiguousarray(v.reshape(s // 8, KAUG, 128))


def _decode_lfT(lfT_core, s):
    """[s/32, T, 512] -> [16, s, T]  (col = 16*(t%32)+b)."""
    v = lfT_core.reshape(s // 32, T, 32, BC1).transpose(3, 0, 2, 1)
    return v.reshape(BC1, s, T)


def _np_lstm(x, mask, W_ih, W_hh, b):
    """Masked LSTM scan, numpy mirror of the reference."""
    Bb, Ss, _ = x.shape
    pre = np.einsum("bsi,gi->bsg", x, W_ih, dtype=np.float32) + b
    h = np.zeros((Bb, H), np.float32)
    c = np.zeros((Bb, H), np.float32)
    ys = np.zeros((Bb, Ss, H), np.float32)
    sig = lambda v: 1.0 / (1.0 + np.exp(-v))
    for t_ in range(Ss):
        g = pre[:, t_] + h @ W_hh.T
        i_, f_, g_, o_ = np.split(g, 4, axis=-1)
        c_new = sig(f_) * c + sig(i_) * np.tanh(g_)
        h_new = sig(o_) * np.tanh(c_new)
        m = mask[:, t_][:, None]
        h = np.where(m, h_new, h)
        c = np.where(m, c_new, c)
        ys[:, t_] = np.where(m, h_new, 0.0)
    return ys


def _np_reference(inputs):
    """Full numpy mirror of the jax reference (fallback path)."""
    x = np.asarray(inputs["input_ids"], np.float32)
    am = np.asarray(inputs["attention_mask"])
    labels = np.asarray(inputs["labels"])
    mask = am > 0
    lengths = am.sum(1)
    h_f = _np_lstm(x, mask, inputs["W_ih_f"], inputs["W_hh_f"], inputs["b_f"])
    t_idx = np.arange(x.shape[1])[None, :]
    rev = np.where(t_idx < lengths[:, None], lengths[:, None] - 1 - t_idx, t_idx)
    x_rev = np.take_along_axis(x, rev[:, :, None], axis=1)
    m_rev = np.take_along_axis(mask, rev, axis=1)
    h_b_rev = _np_lstm(x_rev, m_rev, inputs["W_ih_b"], inputs["W_hh_b"],
                       inputs["b_b"])
    h_b = np.take_along_axis(h_b_rev, rev[:, :, None], axis=1)
    seq = np.concatenate([h_f, h_b], -1)
    logits = seq @ np.asarray(inputs["Wc"], np.float32).T + inputs["bc"]
    return _finish_host(logits, labels, inputs)


def _crf_outputs_np(e, pmask):
    """Numpy CRF forward + viterbi score history (mirror of launch 2)."""
    Bb = e.shape[0]
    s = e.shape[1]
    trans = _crf_trans_global
    score = e[:, 0].copy()
    vscore = np.zeros((Bb, s, T), np.float32)
    vscore[:, 0] = e[:, 0]
    for t_ in range(1, s):
        m = pmask[:, t_][:, None]
        mx = score.max(1, keepdims=True)
        nxt = np.log(np.einsum("bi,ij->bj", np.exp(score - mx),
                               np.exp(trans)).astype(np.float32)) + mx + e[:, t_]
        score = np.where(m, nxt.astype(np.float32), score)
        v = vscore[:, t_ - 1]
        cand = v[:, :, None] + trans[None]
        vb = cand.max(1) + e[:, t_]
        vscore[:, t_] = np.where(m, vb, vscore[:, t_ - 1])
    return score, vscore


def _finish_host(logits, labels, inputs, crf_final=None, vscore=None):
    """Reorder, numerator/denominator/loss, backtrace. Any of crf_final /
    vscore may be None -> computed in numpy."""
    global _crf_trans_global
    start = np.asarray(inputs["crf_start"], np.float32)
    end = np.asarray(inputs["crf_end"], np.float32)
    trans = np.asarray(inputs["crf_trans"], np.float32)
    _crf_trans_global = trans
    Bb, s, _ = logits.shape
    valid = labels >= 0
    order = np.argsort(~valid, axis=1, kind="stable")
    new_logits = np.take_along_axis(logits, order[:, :, None], axis=1)
    new_labels = np.take_along_axis(labels, order, axis=1)
    pmask = np.take_along_axis(valid, order, axis=1)
    active = np.where(pmask, new_labels, 0)

    if crf_final is None or vscore is None:
        e = new_logits.copy()
        e[:, 0] += start
        crf_final, vscore = _crf_outputs_np(e, pmask)

    mf = pmask.astype(np.float32)
    batch = np.arange(Bb)
    emit = np.take_along_axis(new_logits, active[..., None], axis=2)[..., 0]
    num = start[active[:, 0]] + emit[:, 0]
    tr = trans[active[:, :-1], active[:, 1:]]
    num = num + np.sum(mf[:, 1:] * (tr + emit[:, 1:]), axis=1, dtype=np.float32)
    last_idx = pmask.sum(1).astype(np.int64) - 1
    num = num + end[active[batch, last_idx]]

    sc = crf_final + end[None]
    mx = sc.max(1)
    denom = np.log(np.exp(sc - mx[:, None]).sum(1)) + mx
    llh = num - denom
    loss = np.float32(-(llh.sum(dtype=np.float32) / mf.sum(dtype=np.float32)))

    # viterbi backtrace from score history
    last_tag = np.argmax(vscore[:, s - 1] + end[None], axis=1).astype(np.int32)
    tags = np.zeros((Bb, s), np.int32)
    tags[:, s - 1] = last_tag
    tag = last_tag
    for t_ in range(s - 1, 0, -1):
        m = pmask[:, t_]
        cand = vscore[:, t_ - 1] + trans[:, tag].T
        prev = np.argmax(cand, axis=1).astype(np.int32)
        tag = np.where(m, prev, tag)
        tags[:, t_ - 1] = tag
    return loss, logits.astype(np.float32), tags, pmask


_lstm_cache = {}
_crf_cache = {}
_timing_runs = []
_crf_trans_global = None


def kernel(**inputs):
    x = np.ascontiguousarray(np.asarray(inputs["input_ids"], np.float32))
    am = np.asarray(inputs["attention_mask"])
    labels = np.asarray(inputs["labels"], np.int32)
    Bb, s, _ = x.shape
    lengths = am.sum(1).astype(np.int64)
    mask = (am > 0)
    prefix_ok = bool(
        np.all(mask == (np.arange(s)[None, :] < lengths[:, None])))
    if not prefix_ok or Bb != B or s % 64 != 0:
        return _np_reference(inputs)

    maskf = mask.astype(np.float32)
    # reversed inputs for the backward direction
    t_idx = np.arange(s)[None, :]
    rev = np.where(t_idx < lengths[:, None], lengths[:, None] - 1 - t_idx, t_idx)
    x_rev = np.take_along_axis(x, rev[:, :, None], axis=1)
    m_rev = np.take_along_axis(maskf, rev, axis=1)

    Wc = np.asarray(inputs["Wc"], np.float32)
    waT_f = _build_waT(np.asarray(inputs["W_ih_f"], np.float32),
                       np.asarray(inputs["b_f"], np.float32))
    waT_b = _build_waT(np.asarray(inputs["W_ih_b"], np.float32),
                       np.asarray(inputs["b_b"], np.float32))
    p = _gate_perm()
    whT_f = np.ascontiguousarray(
        np.asarray(inputs["W_hh_f"], np.float32)[p].T)
    whT_b = np.ascontiguousarray(
        np.asarray(inputs["W_hh_b"], np.float32)[p].T)
    wcT_f = np.ascontiguousarray(Wc[:, 0:H].T)
    wcT_b = np.ascontiguousarray(Wc[:, H:].T)
    idf = np.eye(128, dtype=np.float32)

    in_maps = []
    for c in range(NCORES):
        fwd = c < 4
        k = c % 4
        sl = slice(BC1 * k, BC1 * (k + 1))
        xs = x[sl] if fwd else x_rev[sl]
        ms = maskf[sl] if fwd else m_rev[sl]
        in_maps.append({
            "xt": _build_xt(xs, ms, s),
            "waT": waT_f if fwd else waT_b,
            "whT": whT_f if fwd else whT_b,
            "wcT": wcT_f if fwd else wcT_b,
            "idf": idf,
        })
    if s not in _lstm_cache:
        _lstm_cache[s] = build_lstm(s)
    _timing_runs.clear()
    _timing_runs.append(("lstm", _lstm_cache[s], in_maps))
    res1 = run_bass_kernel_spmd(_lstm_cache[s], in_maps,
                                core_ids=list(range(NCORES)))

    lf = np.zeros((Bb, s, T), np.float32)
    lb_rev = np.zeros((Bb, s, T), np.float32)
    for c in range(NCORES):
        k = c % 4
        sl = slice(BC1 * k, BC1 * (k + 1))
        dec = _decode_lfT(res1.results[c]["lfT"], s)
        if c < 4:
            lf[sl] = dec
        else:
            lb_rev[sl] = dec
    lb = np.take_along_axis(lb_rev, rev[:, :, None], axis=1)
    logits = lf + lb + np.asarray(inputs["bc"], np.float32)[None, None, :]

    # ---- CRF phase
    start = np.asarray(inputs["crf_start"], np.float32)
    trans = np.asarray(inputs["crf_trans"], np.float32)
    valid = labels >= 0
    order = np.argsort(~valid, axis=1, kind="stable")
    new_logits = np.take_along_axis(logits, order[:, :, None], axis=1)
    pmask = np.take_along_axis(valid, order, axis=1)
    e = new_logits.copy()
    e[:, 0] += start
    ttr = np.tile(np.ascontiguousarray(trans.T).reshape(-1), (BC2, 1))
    etr = np.exp(trans).astype(np.float32)
    in_maps2 = []
    for c in range(NCORES):
        sl = slice(BC2 * c, BC2 * (c + 1))
        in_maps2.append({
            "e": np.ascontiguousarray(e[sl]),
            "ttr": ttr,
            "etr": etr,
            "mask": np.ascontiguousarray(pmask[sl].astype(np.float32)),
            "idf": idf,
        })
    if s not in _crf_cache:
        _crf_cache[s] = build_crf(s)
    _timing_runs.append(("crf", _crf_cache[s], in_maps2))
    res2 = run_bass_kernel_spmd(_crf_cache[s], in_maps2,
                                core_ids=list(range(NCORES)))
    crf_final = np.concatenate(
        [res2.results[c]["crf"] for c in range(NCORES)], axis=0)
    vscore = np.concatenate(
        [res2.results[c]["vhist"] for c in range(NCORES)], axis=0)

    return _finish_host(logits, labels, inputs, crf_final, vscore)
